# revision 11
# baseline (speedup 1.0000x reference)
"""Trainium2 Bass kernel for the BallActor GNN (EdgeConv over fully-connected
per-sample graphs, batch 1024 x 21 objects).

Key algorithmic facts exploited:
  * knn_actor K=20 over NOBJ=21 with self masked => the "kNN graph" is simply
    ALL ordered pairs (i, j != i); top_k is unnecessary and max-aggregation is
    order independent.
  * EdgeConv first layer is linear in [x_i, x_j - x_i]:
        h(i,j) = x_i @ (A - B) + x_j @ B + bm1   (Wm1 = [[A],[B]])
    so per-node terms u_i = x_i@(A-B), v_j = x_j@B are precomputed and each
    edge costs only an elementwise add + tanh + the second 128x128 matmul.
  * The class embedding path collapses to a 3-row table lookup, folded into
    u/v via one-hot rows (host precomputes F3 = tanh(tanh(emb)@We + be) and
    G = F3 @ W_cls); applied as one K=3 matmul against a 3-partition one-hot.
  * Edges are enumerated as 20 cyclic shifts d=1..20: j = (i+d) mod 21.  v is
    stored duplicated along the object axis with an even row stride (42) in
    TWO parity copies so every shift window starts 4B-aligned => the DVE adds
    run in the 2x bf16 perf mode.
  * tanh is monotone, so  max_d tanh(msg_d + bm2) == tanh(max_d msg_d + bm2).
    For BALL_AA shifts the ACT engine evacuates tanh(msg+bm2) from PSUM
    directly and GPSIMD runs those maxes on bf16 SBUF tiles, splitting the
    PSUM-read bottleneck (only DVE/ACT can read PSUM) across both engines.
  * The actor-head output layer is transposed on the PE (node blocks of 84 =
    4 samples x 21 on the partition axis) so the tail activations process
    FD=32 instead of FD=2688 per instruction.

Sharding: pure data parallel over the batch: 1024 samples -> 8 cores x 128.
Params are replicated; outputs are concatenated on host.
"""

import os
import numpy as np
import ml_dtypes

BS = 1024
NOBJ = 21
HID = 128
NCORES = 8
S = BS // NCORES          # samples per core
N = S * NOBJ              # nodes per core (2688)
F32 = np.float32
BF16 = ml_dtypes.bfloat16

# msg psum tiles: two column chunks per shift
MCH = (1344, 1344)
MOFF = (0, 1344)

# weight-pack column layout (single [128, WCOLS] tensor, one DMA)
_OFF_WS2 = 0
_OFF_WUS = 128
_OFF_WVS = 256
_OFF_WM2 = 384
_OFF_WA1 = 512
_OFF_WA2 = 640            # 4 cols
_OFF_WS1 = 644            # [4, 128] on partitions 0..3
_OFF_GU = 644 + 128       # [3, 128] on partitions 0..2
_OFF_GV = _OFF_GU + 128
WCOLS = _OFF_GV + 128

_cache = {}


def _build_nc(edge_dt_name: str):
    import concourse.bass as bass  # noqa: F401
    import concourse.bacc as bacc
    import concourse.tile as tile
    from concourse import mybir

    dt = mybir.dt
    edt = getattr(dt, edge_dt_name)
    AF = mybir.ActivationFunctionType
    OP = mybir.AluOpType

    nc = bacc.Bacc("TRN2")

    # ---------------- DRAM I/O ----------------
    d_state = nc.dram_tensor("state", [S, 63], dt.float32, kind="ExternalInput")
    d_tar = nc.dram_tensor("tar", [S, NOBJ * 2], dt.float32, kind="ExternalInput")
    d_wpack = nc.dram_tensor("wpack", [HID, WCOLS], edt, kind="ExternalInput")
    # bias rows: bs1, bs2, bm1, bm2, ba1, ba2[0], ba2[1], ba2[2], ba2[3]
    d_bias = nc.dram_tensor("biases", [9, HID], dt.float32, kind="ExternalInput")
    # output rows s, cols (h, i, c): h=0 mu, h=1 std; host reshapes
    d_out = nc.dram_tensor("out", [S, 4 * NOBJ], dt.float32, kind="ExternalOutput")

    # ---- shift -> engine assignment ----
    # aa shifts: ACT evacuates tanh(msg+bm2); GPSIMD maxes them in bf16.
    n_aa = int(os.environ.get("BALL_AA", "6"))
    # adds: n_gs on gpsimd, rest on DVE
    n_gs = int(os.environ.get("BALL_GS", "9"))
    LA = int(os.environ.get("BALL_LA", "2"))      # pair lookahead

    shifts = list(range(1, NOBJ))
    # aa shifts spread across the range so GS max work interleaves
    aa_set = set(shifts[::4][:n_aa]) if n_aa else set()
    # gpsimd adds: prefer non-aa shifts spread out
    gs_pool = [d for d in shifts if d not in aa_set]
    step = max(1, len(gs_pool) // max(n_gs, 1))
    gs_set = set(gs_pool[::step][:n_gs]) if n_gs else set()

    pairs = [(shifts[2 * k], shifts[2 * k + 1]) for k in range(len(shifts) // 2)]

    with tile.TileContext(nc) as tc, \
         tc.tile_pool(name="per", bufs=1) as per, \
         tc.tile_pool(name="edge", bufs=3) as edge:

        # ---- persistent tiles ----
        wpack = per.tile([HID, WCOLS], edt, tag="wpack")
        nc.sync.dma_start(out=wpack, in_=d_wpack[:])
        w_Ws2 = wpack[:, _OFF_WS2:_OFF_WS2 + HID]
        w_WuS = wpack[:, _OFF_WUS:_OFF_WUS + HID]
        w_WvS = wpack[:, _OFF_WVS:_OFF_WVS + HID]
        w_Wm2 = wpack[:, _OFF_WM2:_OFF_WM2 + HID]
        w_Wa1 = wpack[:, _OFF_WA1:_OFF_WA1 + HID]
        w_Wa2 = wpack[:, _OFF_WA2:_OFF_WA2 + 4]
        w_Ws1 = wpack[0:4, _OFF_WS1:_OFF_WS1 + HID]
        w_Gu = wpack[0:3, _OFF_GU:_OFF_GU + HID]
        w_Gv = wpack[0:3, _OFF_GV:_OFF_GV + HID]

        # per-partition bias columns [HID, 9]
        bcol = per.tile([HID, 9], dt.float32, tag="bcol")
        nc.sync.dma_start(out=bcol, in_=d_bias[:].rearrange("b h -> h b"))
        bs1 = bcol[:, 0:1]
        bs2 = bcol[:, 1:2]
        bm1 = bcol[:, 2:3]
        bm2 = bcol[:, 3:4]
        ba1 = bcol[:, 4:5]
        ba2c = [bcol[:, 5 + c:6 + c] for c in range(4)]

        cneg = per.tile([HID, 1], dt.float32, tag="cneg")
        nc.vector.memset(cneg, -1.5)

        # BALL_REPEAT>1 re-runs the whole per-inference computation
        # (idempotent) so pipelined-call slope timing isolates device time.
        # Cross-rep state tiles are double-buffered (dbl pool) so rep k+1's
        # phase A can start while rep k is still consuming u/v/agg; a single
        # PSUM pool tag serves all phases so banks recycle without cross-pool
        # conflicts.
        nrep = int(os.environ.get("BALL_REPEAT", "1"))
        with tc.tile_pool(name="dbl", bufs=2) as dbl, \
             tc.tile_pool(name="phA", bufs=2) as phA, \
             tc.tile_pool(name="psB", bufs=2, space="PSUM") as psB, \
             tc.tile_pool(name="psT", bufs=2, space="PSUM") as psT:
          for _rep in range(nrep):
            u_sb = dbl.tile([HID, S, NOBJ], edt, tag="u_sb", name="u_sb")
            # v duplicated along objects, even row stride, two parity copies
            # so every shift window is 4B-aligned (2x bf16 DVE adds)
            v_e = dbl.tile([HID, S, 42], edt, tag="v_e", name="v_e")
            v_o = dbl.tile([HID, S, 42], edt, tag="v_o", name="v_o")
            agg = dbl.tile([HID, N], dt.float32, tag="agg", name="agg")
            if aa_set:
                xa = dbl.tile([HID, N], edt, tag="xa", name="xa")
            else:
                xa = None

            # ---- phase A: inputs -> node features u, v ----

            state_nat = phA.tile([S, 63], dt.float32, tag="state_nat")
            nc.sync.dma_start(out=state_nat, in_=d_state[:])
            tar_nat = phA.tile([S, NOBJ * 2], dt.float32, tag="tar_nat")
            nc.sync.dma_start(out=tar_nat, in_=d_tar[:])

            # tanh(tar) in natural layout (cheap: 42 elems/partition)
            ttar_nat = phA.tile([S, NOBJ * 2], dt.float32, tag="ttar_nat")
            nc.scalar.activation(out=ttar_nat, in_=tar_nat, func=AF.Tanh)

            # one-hot of category in natural layout (exact in bf16)
            oh_nat = phA.tile([S, 3, NOBJ], edt, tag="oh_nat")
            cats_nat = state_nat[:].rearrange("s (i k) -> s k i", k=3)[:, 2, :]
            for c in range(3):
                nc.vector.tensor_scalar(
                    out=oh_nat[:, c, :], in0=cats_nat, scalar1=float(c),
                    scalar2=None, op0=OP.is_equal)

            # Stage spatial channels into a channel-blocked [s, k, i] tile
            # (two cheap in-partition DVE copies, casting to edt) so the
            # partition-collapse DMAs move contiguous 21-element runs.
            st3 = state_nat[:].rearrange("s (i k) -> s k i", k=3)
            tt2 = ttar_nat[:].rearrange("s (i c) -> s c i", c=2)
            comb = phA.tile([S, 4, NOBJ], edt, tag="comb")
            nc.vector.tensor_copy(comb[:, 0:2, :], st3[:, 0:2, :])
            nc.vector.tensor_copy(comb[:, 2:4, :], tt2)
            # channel-major staging: [4, S*NOBJ] and [3, S*NOBJ]
            # (spread across DMA queues so descriptor-gen latencies overlap)
            spat4 = phA.tile([4, S, NOBJ], edt, tag="spat4")
            for c, q in zip(range(4), (nc.sync, nc.gpsimd, nc.sync, nc.gpsimd)):
                q.dma_start(out=spat4[c:c + 1], in_=comb[:, c, :])
            oh3 = phA.tile([3, S, NOBJ], edt, tag="oh3")
            for c, q in zip(range(3), (nc.gpsimd, nc.sync, nc.sync)):
                q.dma_start(out=oh3[c:c + 1], in_=oh_nat[:, c, :])
            spat4f = spat4[:].rearrange("c s i -> c (s i)")
            oh3f = oh3[:].rearrange("c s i -> c (s i)")

            h1 = phA.tile([HID, N], edt, tag="h1")
            feat = phA.tile([HID, N], edt, tag="feat")
            u_f = u_sb[:].rearrange("c s i -> c (s i)")

            # 1344-col halves, psum tiles double-buffered; one ACT/DVE pass
            # per (stage, half).
            for c0, cw in zip(MOFF, MCH):
                p1 = psB.tile([HID, 1344], dt.float32, tag="msg", name="p1")
                for q0 in range(0, cw, 512):
                    qw = min(512, cw - q0)
                    nc.tensor.matmul(p1[:, q0:q0 + qw], w_Ws1,
                                     spat4f[:, c0 + q0:c0 + q0 + qw],
                                     start=True, stop=True)
                nc.scalar.activation(out=h1[:, c0:c0 + cw], in_=p1[:, :cw],
                                     func=AF.Tanh, bias=bs1)
                p2 = psB.tile([HID, 1344], dt.float32, tag="msg", name="p2")
                for q0 in range(0, cw, 512):
                    qw = min(512, cw - q0)
                    nc.tensor.matmul(p2[:, q0:q0 + qw], w_Ws2,
                                     h1[:, c0 + q0:c0 + q0 + qw],
                                     start=True, stop=True)
                nc.scalar.activation(out=feat[:, c0:c0 + cw], in_=p2[:, :cw],
                                     func=AF.Tanh, bias=bs2)
                pu = psB.tile([HID, 1344], dt.float32, tag="msg", name="pu")
                for q0 in range(0, cw, 512):
                    qw = min(512, cw - q0)
                    nc.tensor.matmul(pu[:, q0:q0 + qw], w_WuS,
                                     feat[:, c0 + q0:c0 + q0 + qw],
                                     start=True, stop=False)
                    nc.tensor.matmul(pu[:, q0:q0 + qw], w_Gu,
                                     oh3f[:, c0 + q0:c0 + q0 + qw],
                                     start=False, stop=True)
                nc.vector.tensor_copy(u_f[:, c0:c0 + cw], pu[:, :cw])
                pv = psB.tile([HID, 1344], dt.float32, tag="msg", name="pv")
                for q0 in range(0, cw, 512):
                    qw = min(512, cw - q0)
                    nc.tensor.matmul(pv[:, q0:q0 + qw], w_WvS,
                                     feat[:, c0 + q0:c0 + q0 + qw],
                                     start=True, stop=False)
                    nc.tensor.matmul(pv[:, q0:q0 + qw], w_Gv,
                                     oh3f[:, c0 + q0:c0 + q0 + qw],
                                     start=False, stop=True)
                nc.vector.tensor_copy(
                    v_e[:, c0 // NOBJ:(c0 + cw) // NOBJ, 0:NOBJ],
                    pv[:, :cw].rearrange("c (s i) -> c s i", i=NOBJ))

            # duplicate v columns so every cyclic shift is one strided AP,
            # then build the odd-parity copy (shifted left by one element)
            nc.vector.tensor_copy(v_e[:, :, NOBJ:2 * NOBJ - 1],
                                  v_e[:, :, 0:NOBJ - 1])
            nc.vector.tensor_copy(v_o[:, :, 0:2 * NOBJ - 2],
                                  v_e[:, :, 1:2 * NOBJ - 1])



            # ---- phase B: all 420 edges/sample via 20 cyclic shifts ----
            aa_first = min(aa_set) if aa_set else None
            psum_first = min(d for d in shifts if d not in aa_set)
            t_of = {}

            def produce(pair):
                hp = edge.tile([HID, 2, S, NOBJ], edt, tag="hp")
                for j, d in enumerate(pair):
                    par = d & 1
                    vsrc = v_o if par else v_e
                    c0 = d - par
                    eng = nc.gpsimd if d in gs_set else nc.vector
                    eng.tensor_tensor(
                        out=hp[:, j], in0=u_sb,
                        in1=vsrc[:, :, c0:c0 + NOBJ], op=OP.add)
                tp = edge.tile([HID, 2, N], edt, tag="tp")
                nc.scalar.activation(
                    out=tp[:].rearrange("c a n -> c (a n)"),
                    in_=hp[:].rearrange("c a s i -> c (a s i)"),
                    func=AF.Tanh, bias=bm1)
                t_of[pair] = tp

            def consume(pair):
                tp = t_of.pop(pair)
                for j, d in enumerate(pair):
                    for c0, cw in zip(MOFF, MCH):
                        pm = psB.tile([HID, 1344], dt.float32, tag="msg")
                        for q0 in range(0, cw, 512):
                            qw = min(512, cw - q0)
                            nc.tensor.matmul(
                                pm[:, q0:q0 + qw], w_Wm2,
                                tp[:, j, c0 + q0:c0 + q0 + qw],
                                start=True, stop=True)
                        if d in aa_set:
                            # tanh evac: max commutes with monotone tanh
                            xc = edge.tile([HID, 1344], edt, tag="xc")
                            nc.scalar.activation(
                                out=xc[:, :cw], in_=pm[:, :cw],
                                func=AF.Tanh, bias=bm2)
                            if d == aa_first:
                                nc.vector.tensor_copy(
                                    xa[:, c0:c0 + cw], xc[:, :cw])
                            else:
                                nc.vector.tensor_tensor(
                                    out=xa[:, c0:c0 + cw],
                                    in0=xa[:, c0:c0 + cw],
                                    in1=xc[:, :cw], op=OP.max)
                        elif d == psum_first:
                            nc.vector.tensor_copy(
                                agg[:, c0:c0 + cw], pm[:, :cw])
                        else:
                            nc.vector.tensor_tensor(
                                out=agg[:, c0:c0 + cw],
                                in0=agg[:, c0:c0 + cw],
                                in1=pm[:, :cw], op=OP.max)

            for p in pairs[:LA]:
                produce(p)
            for k, p in enumerate(pairs):
                if k + LA < len(pairs):
                    produce(pairs[k + LA])
                consume(p)
            del t_of

            # ---- phase C: actor head ----
            x = edge.tile([HID, N], edt, tag="hp", name="x")
            a1 = edge.tile([HID, N], edt, tag="tp", name="a1")
            for c0, cw in zip(MOFF, MCH):
                # x = tanh(agg + bm2), merged with the aa branch if present
                if aa_set:
                    xt = edge.tile([HID, 1344], edt, tag="xc", name=f"xt{c0}")
                    nc.scalar.activation(out=xt[:, :cw], in_=agg[:, c0:c0 + cw],
                                         func=AF.Tanh, bias=bm2)
                    nc.vector.tensor_tensor(
                        out=x[:, c0:c0 + cw], in0=xt[:, :cw],
                        in1=xa[:, c0:c0 + cw], op=OP.max)
                else:
                    nc.scalar.activation(out=x[:, c0:c0 + cw],
                                         in_=agg[:, c0:c0 + cw],
                                         func=AF.Tanh, bias=bm2)
                pa = psB.tile([HID, 1344], dt.float32, tag="msg",
                              name=f"pa{c0}")
                for q0 in range(0, cw, 512):
                    qw = min(512, cw - q0)
                    nc.tensor.matmul(pa[:, q0:q0 + qw], w_Wa1,
                                     x[:, c0 + q0:c0 + q0 + qw],
                                     start=True, stop=True)
                nc.scalar.activation(out=a1[:, c0:c0 + cw], in_=pa[:, :cw],
                                     func=AF.Tanh, bias=ba1)

            # transposed output layer: 32 node-blocks of 84 (= 4 samples),
            # partitions become node instances, 4 head outputs per block
            NBLK = 32
            BW = 84
            if True:
                pT = psT.tile([HID, 4 * NBLK], dt.float32, tag="pT")
                for b in range(NBLK):
                    nc.tensor.matmul(pT[0:BW, 4 * b:4 * b + 4],
                                     a1[:, BW * b:BW * (b + 1)], w_Wa2,
                                     start=True, stop=True)
                pT4 = pT[0:BW].rearrange("p (b c) -> p b c", c=4)
                # combined output staging [84p, 32b, 2h, 2c]
                osb = per.tile([HID, NBLK, 2, 2], dt.float32, tag="osb")
                tmu = per.tile([HID, NBLK, 2], edt, tag="tmu")
                for c in range(2):
                    nc.scalar.activation(out=tmu[0:BW, :, c], in_=pT4[:, :, c],
                                         func=AF.Tanh, bias=ba2c[c][0:BW])
                nc.vector.tensor_scalar(
                    out=osb[0:BW, :, 0, :], in0=tmu[0:BW], scalar1=0.3,
                    scalar2=None, op0=OP.mult)
                dview = d_out[:].rearrange("(b q) (h i c) -> q h i b c",
                                           q=4, h=2, c=2)
                for q in range(4):
                    nc.sync.dma_start(out=dview[q, 0],
                                      in_=osb[q * NOBJ:(q + 1) * NOBJ, :, 0, :])
                tls = per.tile([HID, NBLK, 2], edt, tag="tls")
                for c in range(2):
                    nc.scalar.activation(out=tls[0:BW, :, c],
                                         in_=pT4[:, :, 2 + c],
                                         func=AF.Tanh, bias=ba2c[2 + c][0:BW])
                nc.scalar.activation(out=osb[0:BW, :, 1, :], in_=tls[0:BW],
                                     func=AF.Exp, bias=cneg[0:BW], scale=3.5)

                # ---- output DMA (std half; mu half already issued above) ----
                for q in range(4):
                    nc.sync.dma_start(out=dview[q, 1],
                                      in_=osb[q * NOBJ:(q + 1) * NOBJ, :, 1, :])

    nc.finalize()
    return nc


def _prep_params(inputs, edge_np):
    """Host-side pure parameter transforms (weights only, O(param size))."""
    f = lambda k: np.asarray(inputs[k], F32)
    Wm1 = f("Wm1")
    A, B = Wm1[:192], Wm1[192:]
    F3 = np.tanh(np.tanh(f("emb_table")) @ f("We") + f("be"))
    Gu = F3 @ (A[128:] - B[128:])
    Gv = F3 @ B[128:]

    wpack = np.zeros((HID, WCOLS), F32)
    wpack[:, _OFF_WS2:_OFF_WS2 + HID] = f("Ws2")
    wpack[:, _OFF_WUS:_OFF_WUS + HID] = A[:128] - B[:128]
    wpack[:, _OFF_WVS:_OFF_WVS + HID] = B[:128]
    wpack[:, _OFF_WM2:_OFF_WM2 + HID] = f("Wm2")
    wpack[:, _OFF_WA1:_OFF_WA1 + HID] = f("Wa1")
    wpack[:, _OFF_WA2:_OFF_WA2 + 4] = f("Wa2")
    wpack[0:4, _OFF_WS1:_OFF_WS1 + HID] = f("Ws1")
    wpack[0:3, _OFF_GU:_OFF_GU + HID] = Gu
    wpack[0:3, _OFF_GV:_OFF_GV + HID] = Gv

    ba2 = f("ba2")
    biases = np.stack([f("bs1"), f("bs2"), f("bm1"), f("bm2"), f("ba1")]
                      + [np.full(HID, ba2[c], F32) for c in range(4)])
    return dict(
        wpack=np.ascontiguousarray(wpack.astype(edge_np)),
        biases=np.ascontiguousarray(biases),
    )


def kernel(**inputs):
    from concourse.bass_utils import run_bass_kernel_spmd

    edge_dt_name = os.environ.get("BALL_EDGE_DT", "bfloat16")
    trace = os.environ.get("BALL_TRACE", "0") == "1"

    if edge_dt_name not in _cache:
        _cache[edge_dt_name] = _build_nc(edge_dt_name)
    nc = _cache[edge_dt_name]

    edge_np = {"bfloat16": BF16, "float32": F32}[edge_dt_name]
    params = _prep_params(inputs, edge_np)

    state = np.ascontiguousarray(np.asarray(inputs["state_inp"], F32))
    tar = np.asarray(inputs["tar_scores"], F32).reshape(BS, NOBJ * 2)

    in_maps = []
    for c in range(NCORES):
        m = dict(params)
        m["state"] = state[c * S:(c + 1) * S]
        m["tar"] = np.ascontiguousarray(tar[c * S:(c + 1) * S])
        in_maps.append(m)

    res = run_bass_kernel_spmd(nc, in_maps, core_ids=list(range(NCORES)),
                               trace=trace)
    kernel.last_results = res

    outs = [res.results[c]["out"] for c in range(NCORES)]
    full = np.concatenate(outs, axis=0).reshape(BS, 2, NOBJ, 2)
    mu = np.ascontiguousarray(full[:, 0].reshape(BS, 2 * NOBJ))
    std = np.ascontiguousarray(full[:, 1].reshape(BS, 2 * NOBJ))
    return mu, std
